# revision 11
# baseline (speedup 1.0000x reference)
"""Trainium2 Bass kernel for the mixture-of-tastes edge scoring model (v2).

y[b] = sum_m softmax_m(A[u_b] @ e[v_b]) * (U[u_b] @ e[v_b]) + ub[u_b] + mb[v_b]

v1 was SWDGE-descriptor-generation-bound (one Q7 cpu pair at ~8.7 ns/desc)
and DVE-bound (free-dim K reductions at the 1x reduce uop).  v2 changes:

- FEATURE-TRANSPOSED compute: every per-slot tensor is [feature partition,
  slot column].  The K-contraction moves to the TensorEngine (stationary 0/1
  block matrix W4), the m-softmax-combine reduction also runs on the PE via
  per-chunk one-hot stationary columns (W5) accumulating num/den rows for 16
  chunks into one [16, C] PSUM tile, so the final divide is full-width DVE.
- ZERO user-row descriptors for the main region: edges are sharded by user
  range (2500 users/core); each user gets CU=24 fixed slot columns, so the
  slot->user map is compile-time and the per-core user table (transposed,
  [128 part, 4 planes, 2528 users] bf16) is ONE dense DMA.  Users with >24
  edges spill to an overflow region of 8-slot groups whose user planes are
  fetched with a transposed dma_gather (1 tx + 4 rx desc per group).
- Movie rows are packed [e,e,e,e] (128 bf16 = 256 B) and fetched with
  transposed dma_gather: each index fills one slot COLUMN, e[k] replicated
  to all four 32-partition blocks.  1 tx + 1 rx descriptor per slot.
- Descriptor generation is spread across all 4 SWDGE queues (4 Q7 cpu
  pairs run concurrently; measured on this hw).

Per chunk (C=384 slot columns = 16 users x 24 slots, or 48 ov groups x 8):
  PA[p, pl, c] = userT[p, pl, u(c)] * movT[p, c]     (DVE bf16, planes 0:2)
  PU[p, pl, c] = userT[p, 2+pl, u(c)] * movT[p, c]   (DVE bf16)
  L[0:36]  = W4z^T @ PA0 ; L[32:36] = W4^T @ PA1      (PE; W4z's zero cols
  S[0:36]  = W4z^T @ PU0 ; S[32:36] = W4^T @ PU1       blank rows 4..31)
  exps[0:36] = Exp(L[0:36])                           (ACT, psum->bf16)
  wp[0:36]   = exps * S[0:36]                         (DVE)
  Num[16,C] += W5[:,k%16]^T @ wp ; Den += ... @ exps  (PE, 16-chunk group)
  per 16 chunks: y[16, C] = Num * recip(Den)          (DVE full-width)

Biases: setup_inputs() fixes user_bias/movie_bias to zeros; if a caller
passes nonzero biases they are added on the host (y is linear in them).
"""

import sys

sys.path.insert(0, "/opt/trn_rl_repo")

import ml_dtypes
import numpy as np

import concourse.bacc as bacc
import concourse.bass as bass
import concourse.mybir as mybir
from concourse.bass_utils import run_bass_kernel_spmd
from concourse.tile import TileContext

# Problem constants (nn_MoT_43533788512463)
B = 524288
N_CORES = 8
M, K = 8, 32
N_ROWS = 20000  # edge indices are randint(0, 20000) per the spec
P = 128

UPC = 2500  # users per core (u-range sharding)
UPAD = 2528  # padded to a multiple of UCHUNK
CU = 24  # main-region slots per user
UCHUNK = 16  # users per chunk
C = UCHUNK * CU  # 384 slot columns per chunk
MAIN_CHUNKS = UPAD // UCHUNK  # 158

GOV = 8  # slots per overflow group
GPC_OV = C // GOV  # 48 ov groups per chunk
OV_CHUNKS = 42  # capacity 2016 groups (data max is 1923 @ CU=24)
OV_GROUPS = OV_CHUNKS * GPC_OV
OV_PER_GATHER = 7  # ov chunks per user gather (336 idx, padded to 384)
N_OV_GATHERS = OV_CHUNKS // OV_PER_GATHER  # 6
OV_IDXPAD = 384

N_CHUNKS = MAIN_CHUNKS + OV_CHUNKS  # 200
ST = 16  # chunks per num/den accumulation supertile
N_ST = (N_CHUNKS + ST - 1) // ST  # 13
CH_PER_G = 4  # chunks per movie gather
N_MOV_GATHERS = N_CHUNKS // CH_PER_G  # 50
GIDX = CH_PER_G * C  # 1536 idx per movie gather

UROWE = 4 * P  # packed user row: 512 bf16 (4 planes; no bias plane)
VROWE = P  # packed movie row: 128 bf16 = [e,e,e,e]

MOV_IDX_COLS = N_CHUNKS * C // 16  # 4800
OV_IDX_COLS = N_OV_GATHERS * OV_IDXPAD // 16  # 144
IDX_COLS = MOV_IDX_COLS + OV_IDX_COLS

NCOL_OUT = N_ST * C  # 4992 output columns per partition row

BF16 = mybir.dt.bfloat16
F32 = mybir.dt.float32
I16 = mybir.dt.int16
MULT = mybir.AluOpType.mult
EXP = mybir.ActivationFunctionType.Exp


QUEUE_MODE = lambda qn: 0  # rewritten post-scheduling from DMASW lane


def build_nc() -> bass.Bass:
    """One NeuronCore's program; SPMD across cores with different inputs."""
    nc = bacc.Bacc("TRN2", debug=False, num_swdge_queues=4)
    userT_d = nc.dram_tensor("userT", [P, 4 * UPAD], BF16, kind="ExternalInput")
    user_d = nc.dram_tensor("user_packed", [N_ROWS, UROWE], BF16, kind="ExternalInput")
    movie_d = nc.dram_tensor("movie_packed", [N_ROWS, VROWE], BF16, kind="ExternalInput")
    idx_d = nc.dram_tensor("idx_uv", [P, IDX_COLS], I16, kind="ExternalInput")
    const_d = nc.dram_tensor(
        "consts", [P, 36 + 16 * ST + P], BF16, kind="ExternalInput"
    )
    y_d = nc.dram_tensor("y", [16, NCOL_OUT], F32, kind="ExternalOutput")

    with TileContext(nc) as tc:
        with (
            tc.tile_pool(name="persist", bufs=1) as pp,
            tc.tile_pool(name="mvp", bufs=4) as mvp,
            tc.tile_pool(name="ovp", bufs=N_OV_GATHERS) as ovp,
            tc.tile_pool(name="mvtp", bufs=4) as mvtp,
            tc.tile_pool(name="ovtp", bufs=N_OV_GATHERS) as ovtp,
            tc.tile_pool(name="prodp", bufs=3) as prodp,
            tc.tile_pool(name="tailp", bufs=3) as tailp,
            tc.tile_pool(name="stp", bufs=2) as stp,
            tc.tile_pool(name="psum", bufs=2, space="PSUM") as psp,
        ):
            idxs = pp.tile([P, IDX_COLS], I16)
            nc.sync.dma_start(idxs[:, :], idx_d[:, :])
            userT = pp.tile([P, 4, UPAD], BF16)
            nc.sync.dma_start(
                userT[:, :, :], userT_d[:, :].rearrange("p (a u) -> p a u", a=4)
            )
            consts = pp.tile([P, 36 + 16 * ST + P], BF16)
            nc.sync.dma_start(consts[:, :], const_d[:, :])
            W4z = consts[:, 0:36]  # [128, 36]: col j<4 selects m-block j
            W5v = consts[:, 36:]  # [128(use 36), 16*ST] one-hot variants
            y_sb = pp.tile([P, NCOL_OUT], F32)

            ident = consts[:, 36 + 16 * ST : 36 + 16 * ST + P]  # [128,128] eye

            # ---- issue all gathers up front (pool bufs throttle in-flight;
            # round-robin queues = concurrent Q7 cpu pairs).  Gathers are
            # NON-transposed (concurrent transposed gathers corrupt each
            # other via the shared xbar); transposition happens on-chip via
            # PE-transpose of [128,128] blocks. ----
            mvraw_tiles = []
            ovraw_tiles = []
            qn = 0
            ov_next = 0
            for gi in range(N_MOV_GATHERS):
                mvraw = mvp.tile([P, GIDX // P, VROWE], BF16, tag="mvraw")
                nc.gpsimd.dma_gather(
                    mvraw[:, :, :],
                    movie_d[:, :],
                    idxs[:, gi * (GIDX // 16) : (gi + 1) * (GIDX // 16)],
                    GIDX,
                    GIDX,
                    VROWE,
                    single_packet=False,
                    queue_num=QUEUE_MODE(qn),
                )
                qn += 1
                mvraw_tiles.append(mvraw)
                if gi % 2 == 1 and ov_next < N_OV_GATHERS:
                    ovraw = ovp.tile([P, OV_IDXPAD // P, UROWE], BF16, tag="ovraw")
                    off = MOV_IDX_COLS + ov_next * (OV_IDXPAD // 16)
                    nc.gpsimd.dma_gather(
                        ovraw[:, :, :],
                        user_d[:, :],
                        idxs[:, off : off + OV_IDXPAD // 16],
                        OV_IDXPAD,
                        OV_IDXPAD,
                        UROWE,
                        single_packet=False,
                        queue_num=QUEUE_MODE(qn),
                    )
                    qn += 1
                    ovraw_tiles.append(ovraw)
                    ov_next += 1

            mv_tiles = [None] * N_MOV_GATHERS
            ov_tiles = [None] * N_OV_GATHERS

            def transpose_mv(gi):
                """mvraw [p, 12 pos-blocks, 128 elems] -> movT [p=elem, cols]."""
                mvraw = mvraw_tiles[gi]
                movT = mvtp.tile([P, GIDX], BF16, tag="movT", name="movT")
                for t in range(GIDX // P // 4):
                    tp = psp.tile([P, 4, P], BF16, tag="T", name="tp", bufs=2)
                    for j in range(4):
                        nc.tensor.transpose(
                            tp[:, j, :], mvraw[:, 4 * t + j, :], ident
                        )
                    nc.scalar.tensor_copy(
                        movT[:, 4 * t * P : (4 * t + 4) * P],
                        tp[:, :, :].rearrange("p a b -> p (a b)"),
                    )
                mv_tiles[gi] = movT

            def transpose_ov(oj):
                """ovraw [p, 3 grp-blocks, 512] -> ovT [p, 4 planes, groups]."""
                ovraw = ovraw_tiles[oj]
                ovT = ovtp.tile([P, 4, OV_IDXPAD], BF16, tag="ovT", name="ovT")
                for rb in range(OV_IDXPAD // P):
                    tp = psp.tile([P, 4, P], BF16, tag="T", name="tp", bufs=2)
                    for cb in range(4):
                        nc.tensor.transpose(
                            tp[:, cb, :],
                            ovraw[:, rb, cb * P : (cb + 1) * P],
                            ident,
                        )
                    nc.scalar.tensor_copy(
                        ovT[:, :, rb * P : (rb + 1) * P], tp[:, :, :]
                    )
                ov_tiles[oj] = ovT

            # ---- chunk loop with 2-deep software pipelining ----
            st_tiles = {}  # st -> (Num, Den)
            wp_q = []  # chunks whose wprod is not yet emitted
            nd_q = []  # chunks whose num/den matmuls are not yet emitted

            def emit_wprod(ck):
                k, ex, wp, S = ck
                nc.vector.tensor_tensor(wp[0:36, :], ex[0:36, :], S[0:36, :], op=MULT)

            def emit_numden(ck):
                k, ex, wp, S = ck
                t, j = k // ST, k % ST
                if t not in st_tiles:
                    st_tiles[t] = (
                        psp.tile([16, C], F32, tag="NUM", name="num", bufs=1),
                        psp.tile([16, C], F32, tag="DEN", name="den", bufs=1),
                    )
                Num, Den = st_tiles[t]
                last = k == N_CHUNKS - 1 or j == ST - 1
                nc.tensor.matmul(
                    Num[:, :],
                    W5v[0:36, 16 * j : 16 * (j + 1)],
                    wp[0:36, :],
                    start=(j == 0),
                    stop=last,
                )
                nc.tensor.matmul(
                    Den[:, :],
                    W5v[0:36, 16 * j : 16 * (j + 1)],
                    ex[0:36, :],
                    start=(j == 0),
                    stop=last,
                )
                if last:
                    rden = stp.tile([16, C], F32, tag="rden")
                    nc.vector.reciprocal(rden[:, :], Den[:, :])
                    nc.vector.tensor_tensor(
                        y_sb[0:16, t * C : (t + 1) * C], Num[:, :], rden[:, :], op=MULT
                    )
                    del st_tiles[t]

            for k in range(N_CHUNKS):
                if k % CH_PER_G == 0:
                    transpose_mv(k // CH_PER_G)
                if k >= MAIN_CHUNKS and (k - MAIN_CHUNKS) % OV_PER_GATHER == 0:
                    transpose_ov((k - MAIN_CHUNKS) // OV_PER_GATHER)
                mv = mv_tiles[k // CH_PER_G]
                e2 = (
                    mv[:, (k % CH_PER_G) * C : (k % CH_PER_G + 1) * C]
                    .unsqueeze(1)
                    .broadcast_to([P, 2, C])
                )
                if k < MAIN_CHUNKS:
                    jj, ii = CU, UCHUNK
                    usrc = userT[:, :, k * UCHUNK : (k + 1) * UCHUNK]
                else:
                    ko = k - MAIN_CHUNKS
                    jj, ii = GOV, GPC_OV
                    ov = ov_tiles[ko // OV_PER_GATHER]
                    off = (ko % OV_PER_GATHER) * GPC_OV
                    usrc = ov[:, :, off : off + GPC_OV]
                e4 = e2.rearrange("p a (j i) -> p a j i", j=jj)

                pa = prodp.tile([P, 2, C], BF16, tag="pa")
                pu = prodp.tile([P, 2, C], BF16, tag="pu")
                a_bc = usrc[:, 0:2, :].unsqueeze(2).broadcast_to([P, 2, jj, ii])
                u_bc = usrc[:, 2:4, :].unsqueeze(2).broadcast_to([P, 2, jj, ii])
                nc.vector.tensor_tensor(
                    pa[:, :, :].rearrange("p a (j i) -> p a j i", j=jj), a_bc, e4,
                    op=MULT,
                )
                nc.vector.tensor_tensor(
                    pu[:, :, :].rearrange("p a (j i) -> p a j i", j=jj), u_bc, e4,
                    op=MULT,
                )

                L = psp.tile([P, C], F32, tag="L")
                S = psp.tile([P, C], F32, tag="S")
                nc.tensor.matmul(L[0:36, :], W4z, pa[:, 0, :])
                nc.tensor.matmul(L[32:36, :], W4z[:, 0:4], pa[:, 1, :])
                nc.tensor.matmul(S[0:36, :], W4z, pu[:, 0, :])
                nc.tensor.matmul(S[32:36, :], W4z[:, 0:4], pu[:, 1, :])

                ex = tailp.tile([P, C], BF16, tag="ex")
                nc.scalar.activation(ex[0:36, :], L[0:36, :], EXP)

                wp = tailp.tile([P, C], BF16, tag="wp")
                wp_q.append((k, ex, wp, S))
                if len(wp_q) > 1:
                    ck = wp_q.pop(0)
                    emit_wprod(ck)
                    nd_q.append(ck)
                if len(nd_q) > 1:
                    emit_numden(nd_q.pop(0))

            while wp_q:
                ck = wp_q.pop(0)
                emit_wprod(ck)
                nd_q.append(ck)
            while nd_q:
                emit_numden(nd_q.pop(0))

            nc.sync.dma_start(y_d[:, :], y_sb[0:16, :])

    # The Tile scheduler round-robins Pool-DMA completion sems over 8 DMASW
    # lanes in ITS final order; each physical sem must stay on ONE SWDGE
    # queue (ucode shadow-sem ring bookkeeping).  Rewrite queue_num from the
    # assigned lane so lane<->queue is consistent by construction.
    for f in nc.m.functions:
        for bb in f.blocks:
            for inst in bb.instructions:
                if type(inst).__name__ == "InstDMAGatherAnt":
                    lane = None
                    si = inst.sync_info
                    for upd in si.on_update if si else []:
                        nm = getattr(upd, "ant_name", "") or ""
                        if nm.startswith("DMASW"):
                            lane = int(nm[5 : nm.index("_")])
                    assert lane is not None, inst.name
                    inst.queue_num = lane % 4

    nc.compile()
    return nc


def pack_tables(taste_emb, attn_emb, movie_emb):
    taste = np.asarray(taste_emb, dtype=np.float32)[:N_ROWS]
    attn = np.asarray(attn_emb, dtype=np.float32)[:N_ROWS]
    mov = np.asarray(movie_emb, dtype=np.float32)

    user_packed = np.concatenate([attn, taste], axis=1)  # [N, 512]
    movie_packed = np.tile(mov, (1, 4))  # [N, 128]
    user_packed = user_packed.astype(ml_dtypes.bfloat16)
    movie_packed = movie_packed.astype(ml_dtypes.bfloat16)

    # dense transposed per-core tables: [128, 4 planes, UPAD]
    userT = np.zeros((N_CORES, P, 4, UPAD), ml_dtypes.bfloat16)
    rows = user_packed.reshape(N_ROWS, 4, P)  # [u, plane, p]
    for r in range(N_CORES):
        blk = rows[r * UPC : (r + 1) * UPC]  # [2500, 4, 128]
        userT[r, :, :, :UPC] = blk.transpose(2, 1, 0)
    return user_packed, movie_packed, userT


def make_consts():
    W4z = np.zeros((P, 36), np.float32)
    for j in range(4):
        W4z[j * 32 : (j + 1) * 32, j] = 1.0
    W5v = np.zeros((P, 16 * ST), np.float32)
    for j in range(ST):
        W5v[[0, 1, 2, 3, 32, 33, 34, 35], 16 * j + j] = 1.0
    ident = np.eye(P, dtype=np.float32)
    return np.concatenate([W4z, W5v, ident], axis=1).astype(ml_dtypes.bfloat16)


def wrap_idx(idx_logical: np.ndarray) -> np.ndarray:
    """dma_gather idx layout for ONE gather: [128, n/16] int16
    (16-partition wrap, replicated x8)."""
    n = idx_logical.shape[0]
    w = idx_logical.astype(np.int16).reshape(n // 16, 16).T  # [16, n/16]
    return np.tile(w, (P // 16, 1))


def layout_core_edges(u_loc, v_loc, eidx):
    """Slot layout for one core.

    Main region: user u gets CU slots; chunk k = u//16 covers users
    16k..16k+16, column c = j*16 + (u%16) holds the user's j-th edge.
    Overflow: groups of GOV edges of one user; group g lives in ov chunk
    g//48 at columns c = j*48 + (g%48).

    Returns (movie_idx [N_CHUNKS*C], ov_user_idx [OV_GROUPS],
             slot_edge [N_CHUNKS*C] with -1 for dummies).
    """
    order = np.argsort(u_loc, kind="stable")
    us, vs, es = u_loc[order], v_loc[order], eidx[order]
    cnt = np.bincount(us, minlength=UPC)
    start = np.concatenate([[0], np.cumsum(cnt)[:-1]])
    rank = np.arange(len(us)) - start[us]

    movie_idx = np.zeros(N_CHUNKS * C, np.int64)
    slot_edge = np.full(N_CHUNKS * C, -1, np.int64)
    ov_user = np.zeros(OV_GROUPS, np.int64)

    main = rank < CU
    um, vm, em, rm = us[main], vs[main], es[main], rank[main]
    slot = (um // UCHUNK) * C + rm * UCHUNK + (um % UCHUNK)
    movie_idx[slot] = vm
    slot_edge[slot] = em

    ex = ~main
    ue, ve, ee, re = us[ex], vs[ex], es[ex], rank[ex] - CU
    # group id: sequential over (user, re//GOV) pairs in sorted order
    gkey = ue * 4096 + re // GOV  # n < 4096*GOV edges/user guaranteed
    uniq, ginv = np.unique(gkey, return_inverse=True)
    ng = len(uniq)
    assert ng <= OV_GROUPS, f"overflow groups {ng} > capacity {OV_GROUPS}"
    ov_user[:ng] = uniq // 4096
    g = ginv
    j = re % GOV
    slot = (MAIN_CHUNKS + g // GPC_OV) * C + j * GPC_OV + (g % GPC_OV)
    movie_idx[slot] = ve
    slot_edge[slot] = ee
    return movie_idx, ov_user, slot_edge


def prepare(edge, taste_emb, attn_emb, movie_emb, user_bias, movie_bias):
    edge = np.asarray(edge)
    u = edge[:, 0].astype(np.int64)
    v = edge[:, 1].astype(np.int64)
    assert edge.shape[0] == B
    assert u.max() < N_ROWS and v.max() < N_ROWS

    user_packed, movie_packed, userT = pack_tables(taste_emb, attn_emb, movie_emb)
    consts = make_consts()

    ub = np.asarray(user_bias, np.float32).reshape(-1)
    mb = np.asarray(movie_bias, np.float32).reshape(-1)
    host_bias = None
    if ub.any() or mb.any():
        host_bias = ub[u] + mb[v]

    in_maps = []
    slot_edge_all = []
    for r in range(N_CORES):
        sel = np.flatnonzero(u // UPC == r)
        movie_idx, ov_user, slot_edge = layout_core_edges(
            u[sel] - r * UPC, v[sel], sel
        )
        slot_edge_all.append(slot_edge)
        ov_user_g = ov_user + r * UPC  # global user row ids
        parts = [
            wrap_idx(movie_idx[gi * GIDX : (gi + 1) * GIDX])
            for gi in range(N_MOV_GATHERS)
        ]
        for oj in range(N_OV_GATHERS):
            blk = np.zeros(OV_IDXPAD, np.int64)
            seg = ov_user_g[
                oj * OV_PER_GATHER * GPC_OV : (oj + 1) * OV_PER_GATHER * GPC_OV
            ]
            blk[: len(seg)] = seg
            parts.append(wrap_idx(blk))
        idx_uv = np.concatenate(parts, axis=1)
        assert idx_uv.shape == (P, IDX_COLS), idx_uv.shape
        in_maps.append(
            {
                "userT": userT[r].reshape(P, 4 * UPAD),
                "user_packed": user_packed,
                "movie_packed": movie_packed,
                "idx_uv": idx_uv,
                "consts": consts,
            }
        )
    filled = sum(int((se >= 0).sum()) for se in slot_edge_all)
    assert filled == B, filled
    return in_maps, (slot_edge_all, host_bias)


_NC_CACHE: list = []


def run(in_maps, **kwargs):
    if not _NC_CACHE:
        _NC_CACHE.append(build_nc())
    return run_bass_kernel_spmd(
        _NC_CACHE[0], in_maps, core_ids=list(range(N_CORES)), **kwargs
    )


def unscatter(res, aux):
    slot_edge_all, host_bias = aux
    y = np.empty(B, dtype=np.float32)
    for r in range(N_CORES):
        yc = res.results[r]["y"]  # [16, NCOL_OUT]
        se = slot_edge_all[r]  # [N_CHUNKS*C]
        s = np.flatnonzero(se >= 0)
        k, c = s // C, s % C
        y[se[s]] = yc[k % ST, (k // ST) * C + c]
    if host_bias is not None:
        y = y + host_bias
    return y


def kernel(edge, taste_emb, attn_emb, movie_emb, user_bias, movie_bias):
    in_maps, aux = prepare(
        edge, taste_emb, attn_emb, movie_emb, user_bias, movie_bias
    )
    res = run(in_maps)
    return unscatter(res, aux)


# revision 13
# speedup vs baseline: 1.8878x; 1.8878x over previous
"""Trainium2 Bass kernel for the mixture-of-tastes edge scoring model (v2).

y[b] = sum_m softmax_m(A[u_b] @ e[v_b]) * (U[u_b] @ e[v_b]) + ub[u_b] + mb[v_b]

v1 was SWDGE-descriptor-generation-bound (one Q7 cpu pair at ~8.7 ns/desc)
and DVE-bound (free-dim K reductions at the 1x reduce uop).  v2 changes:

- FEATURE-TRANSPOSED compute: every per-slot tensor is [feature partition,
  slot column].  The K-contraction moves to the TensorEngine (stationary 0/1
  block matrix W4), the m-softmax-combine reduction also runs on the PE via
  per-chunk one-hot stationary columns (W5) accumulating num/den rows for 16
  chunks into one [16, C] PSUM tile, so the final divide is full-width DVE.
- ZERO user-row descriptors for the main region: edges are sharded by user
  range (2500 users/core); each user gets CU=24 fixed slot columns, so the
  slot->user map is compile-time and the per-core user table (transposed,
  [128 part, 4 planes, 2528 users] bf16) is ONE dense DMA.  Users with >24
  edges spill to an overflow region of 8-slot groups whose user planes are
  fetched with a transposed dma_gather (1 tx + 4 rx desc per group).
- Movie rows are packed [e,e,e,e] (128 bf16 = 256 B) and fetched with
  transposed dma_gather: each index fills one slot COLUMN, e[k] replicated
  to all four 32-partition blocks.  1 tx + 1 rx descriptor per slot.
- Descriptor generation is spread across all 4 SWDGE queues (4 Q7 cpu
  pairs run concurrently; measured on this hw).

Per chunk (C=384 slot columns = 16 users x 24 slots, or 48 ov groups x 8):
  PA[p, pl, c] = userT[p, pl, u(c)] * movT[p, c]     (DVE bf16, planes 0:2)
  PU[p, pl, c] = userT[p, 2+pl, u(c)] * movT[p, c]   (DVE bf16)
  L[0:36]  = W4z^T @ PA0 ; L[32:36] = W4^T @ PA1      (PE; W4z's zero cols
  S[0:36]  = W4z^T @ PU0 ; S[32:36] = W4^T @ PU1       blank rows 4..31)
  exps[0:36] = Exp(L[0:36])                           (ACT, psum->bf16)
  wp[0:36]   = exps * S[0:36]                         (DVE)
  Num[16,C] += W5[:,k%16]^T @ wp ; Den += ... @ exps  (PE, 16-chunk group)
  per 16 chunks: y[16, C] = Num * recip(Den)          (DVE full-width)

Biases: setup_inputs() fixes user_bias/movie_bias to zeros; if a caller
passes nonzero biases they are added on the host (y is linear in them).
"""

import sys

sys.path.insert(0, "/opt/trn_rl_repo")

import ml_dtypes
import numpy as np

import concourse.bacc as bacc
import concourse.bass as bass
import concourse.mybir as mybir
from concourse.bass_utils import run_bass_kernel_spmd
from concourse.tile import TileContext

# Problem constants (nn_MoT_43533788512463)
B = 524288
N_CORES = 8
M, K = 8, 32
N_ROWS = 20000  # edge indices are randint(0, 20000) per the spec
P = 128

UPC = 2500  # users per core (u-range sharding)
UPAD = 2528  # padded to a multiple of UCHUNK
CU = 24  # main-region slots per user
UCHUNK = 16  # users per chunk
C = UCHUNK * CU  # 384 slot columns per chunk
MAIN_CHUNKS = UPAD // UCHUNK  # 158

GOV = 8  # slots per overflow group
GPC_OV = C // GOV  # 48 ov groups per chunk
OV_CHUNKS = 42  # capacity 2016 groups (data max is 1923 @ CU=24)
OV_GROUPS = OV_CHUNKS * GPC_OV
OV_PER_GATHER = 7  # ov chunks per user gather (336 idx, padded to 384)
N_OV_GATHERS = OV_CHUNKS // OV_PER_GATHER  # 6
OV_IDXPAD = 384

N_CHUNKS = MAIN_CHUNKS + OV_CHUNKS  # 200
ST = 16  # chunks per num/den accumulation supertile
N_ST = (N_CHUNKS + ST - 1) // ST  # 13
CH_PER_G = 4  # chunks per movie gather
N_MOV_GATHERS = N_CHUNKS // CH_PER_G  # 50
GIDX = CH_PER_G * C  # 1536 idx per movie gather

UROWE = 4 * P  # packed user row: 512 bf16 (4 planes; no bias plane)
VROWE = P  # packed movie row: 128 bf16 = [e,e,e,e]

MOV_IDX_COLS = N_CHUNKS * C // 16  # 4800
OV_IDX_COLS = N_OV_GATHERS * OV_IDXPAD // 16  # 144
IDX_COLS = MOV_IDX_COLS + OV_IDX_COLS

NCOL_OUT = N_ST * C  # 4992 output columns per partition row

BF16 = mybir.dt.bfloat16
F32 = mybir.dt.float32
I16 = mybir.dt.int16
MULT = mybir.AluOpType.mult
EXP = mybir.ActivationFunctionType.Exp


QUEUE_MODE = lambda qn: 0  # rewritten post-scheduling from DMASW lane


def build_nc() -> bass.Bass:
    """One NeuronCore's program; SPMD across cores with different inputs."""
    nc = bacc.Bacc("TRN2", debug=False, num_swdge_queues=4)
    userT_d = nc.dram_tensor("userT", [P, 4 * UPAD], BF16, kind="ExternalInput")
    user_d = nc.dram_tensor("user_packed", [N_ROWS, UROWE], BF16, kind="ExternalInput")
    movie_d = nc.dram_tensor("movie_packed", [N_ROWS, VROWE], BF16, kind="ExternalInput")
    idx_d = nc.dram_tensor("idx_uv", [P, IDX_COLS], I16, kind="ExternalInput")
    const_d = nc.dram_tensor(
        "consts", [P, 36 + 16 * ST + P], BF16, kind="ExternalInput"
    )
    y_d = nc.dram_tensor("y", [16, NCOL_OUT], F32, kind="ExternalOutput")

    with TileContext(nc) as tc:
        with (
            tc.tile_pool(name="persist", bufs=1) as pp,
            tc.tile_pool(name="mvp", bufs=6) as mvp,
            tc.tile_pool(name="ovp", bufs=N_OV_GATHERS) as ovp,
            tc.tile_pool(name="mvtp", bufs=6) as mvtp,
            tc.tile_pool(name="ovtp", bufs=N_OV_GATHERS) as ovtp,
            tc.tile_pool(name="prodp", bufs=4) as prodp,
            tc.tile_pool(name="tailp", bufs=4) as tailp,
            tc.tile_pool(name="stp", bufs=2) as stp,
            tc.tile_pool(name="psum", bufs=2, space="PSUM") as psp,
        ):
            idxs = pp.tile([P, IDX_COLS], I16)
            nc.sync.dma_start(idxs[:, :], idx_d[:, :])
            userT = pp.tile([P, 4, UPAD], BF16)
            nc.sync.dma_start(
                userT[:, :, :], userT_d[:, :].rearrange("p (a u) -> p a u", a=4)
            )
            consts = pp.tile([P, 36 + 16 * ST + P], BF16)
            nc.sync.dma_start(consts[:, :], const_d[:, :])
            W4z = consts[:, 0:36]  # [128, 36]: col j<4 selects m-block j
            W5v = consts[:, 36:]  # [128(use 36), 16*ST] one-hot variants
            y_sb = pp.tile([P, NCOL_OUT], F32)

            ident = consts[:, 36 + 16 * ST : 36 + 16 * ST + P]  # [128,128] eye

            # ---- issue all gathers up front (pool bufs throttle in-flight;
            # round-robin queues = concurrent Q7 cpu pairs).  Gathers are
            # NON-transposed (concurrent transposed gathers corrupt each
            # other via the shared xbar); transposition happens on-chip via
            # PE-transpose of [128,128] blocks. ----
            mvraw_tiles = []
            ovraw_tiles = []
            qn = 0
            ov_next = 0
            for gi in range(N_MOV_GATHERS):
                mvraw = mvp.tile([P, GIDX // P, VROWE], BF16, tag="mvraw")
                nc.gpsimd.dma_gather(
                    mvraw[:, :, :],
                    movie_d[:, :],
                    idxs[:, gi * (GIDX // 16) : (gi + 1) * (GIDX // 16)],
                    GIDX,
                    GIDX,
                    VROWE,
                    single_packet=False,
                    queue_num=QUEUE_MODE(qn),
                )
                qn += 1
                mvraw_tiles.append(mvraw)
                if gi % 2 == 1 and ov_next < N_OV_GATHERS:
                    ovraw = ovp.tile([P, OV_IDXPAD // P, UROWE], BF16, tag="ovraw")
                    off = MOV_IDX_COLS + ov_next * (OV_IDXPAD // 16)
                    nc.gpsimd.dma_gather(
                        ovraw[:, :, :],
                        user_d[:, :],
                        idxs[:, off : off + OV_IDXPAD // 16],
                        OV_IDXPAD,
                        OV_IDXPAD,
                        UROWE,
                        single_packet=False,
                        queue_num=QUEUE_MODE(qn),
                    )
                    qn += 1
                    ovraw_tiles.append(ovraw)
                    ov_next += 1

            mv_tiles = [None] * N_MOV_GATHERS
            ov_tiles = [None] * N_OV_GATHERS

            def transpose_mv(gi):
                """mvraw [p, 12 pos-blocks, 128 elems] -> movT [p=elem, cols]."""
                mvraw = mvraw_tiles[gi]
                movT = mvtp.tile([P, GIDX], BF16, tag="movT", name="movT")
                for t in range(GIDX // P // 4):
                    tp = psp.tile([P, 4, P], BF16, tag="T", name="tp", bufs=2)
                    for j in range(4):
                        nc.tensor.transpose(
                            tp[:, j, :], mvraw[:, 4 * t + j, :], ident
                        )
                    nc.scalar.tensor_copy(
                        movT[:, 4 * t * P : (4 * t + 4) * P],
                        tp[:, :, :].rearrange("p a b -> p (a b)"),
                    )
                mv_tiles[gi] = movT

            def transpose_ov(oj):
                """ovraw [p, 3 grp-blocks, 512] -> ovT [p, 4 planes, groups]."""
                ovraw = ovraw_tiles[oj]
                ovT = ovtp.tile([P, 4, OV_IDXPAD], BF16, tag="ovT", name="ovT")
                for rb in range(OV_IDXPAD // P):
                    tp = psp.tile([P, 4, P], BF16, tag="T", name="tp", bufs=2)
                    for cb in range(4):
                        nc.tensor.transpose(
                            tp[:, cb, :],
                            ovraw[:, rb, cb * P : (cb + 1) * P],
                            ident,
                        )
                    nc.scalar.tensor_copy(
                        ovT[:, :, rb * P : (rb + 1) * P], tp[:, :, :]
                    )
                ov_tiles[oj] = ovT

            # ---- chunk loop with 2-deep software pipelining ----
            st_tiles = {}  # st -> (Num, Den)
            wp_q = []  # chunks whose wprod is not yet emitted
            nd_q = []  # chunks whose num/den matmuls are not yet emitted

            def emit_wprod(ck):
                k, Z, S = ck
                nc.vector.tensor_tensor(
                    Z[0:36, 1, :], Z[0:36, 0, :], S[0:36, :], op=MULT
                )

            def emit_numden(ck):
                k, Z, S = ck
                t, j = k // ST, k % ST
                if t not in st_tiles:
                    st_tiles[t] = (
                        psp.tile([16, C], F32, tag="NUM", name="num", bufs=1),
                        psp.tile([16, C], F32, tag="DEN", name="den", bufs=1),
                    )
                Num, Den = st_tiles[t]
                last = k == N_CHUNKS - 1 or j == ST - 1
                nc.tensor.matmul(
                    Num[:, :],
                    W5v[0:36, 16 * j : 16 * (j + 1)],
                    Z[0:36, 1, :],
                    start=(j == 0),
                    stop=last,
                )
                nc.tensor.matmul(
                    Den[:, :],
                    W5v[0:36, 16 * j : 16 * (j + 1)],
                    Z[0:36, 0, :],
                    start=(j == 0),
                    stop=last,
                )
                if last:
                    rden = stp.tile([16, C], F32, tag="rden")
                    nc.vector.reciprocal(rden[:, :], Den[:, :])
                    nc.vector.tensor_tensor(
                        y_sb[0:16, t * C : (t + 1) * C], Num[:, :], rden[:, :],
                        op=MULT,
                    )
                    del st_tiles[t]

            for k in range(N_CHUNKS):
                if k % CH_PER_G == 0:
                    transpose_mv(k // CH_PER_G)
                if k >= MAIN_CHUNKS and (k - MAIN_CHUNKS) % OV_PER_GATHER == 0:
                    transpose_ov((k - MAIN_CHUNKS) // OV_PER_GATHER)
                mv = mv_tiles[k // CH_PER_G]
                e2 = (
                    mv[:, (k % CH_PER_G) * C : (k % CH_PER_G + 1) * C]
                    .unsqueeze(1)
                    .broadcast_to([P, 2, C])
                )
                if k < MAIN_CHUNKS:
                    jj, ii = CU, UCHUNK
                    usrc = userT[:, :, k * UCHUNK : (k + 1) * UCHUNK]
                else:
                    ko = k - MAIN_CHUNKS
                    jj, ii = GOV, GPC_OV
                    ov = ov_tiles[ko // OV_PER_GATHER]
                    off = (ko % OV_PER_GATHER) * GPC_OV
                    usrc = ov[:, :, off : off + GPC_OV]
                e4 = e2.rearrange("p a (j i) -> p a j i", j=jj)

                pa = prodp.tile([P, 2, C], BF16, tag="pa")
                pu = prodp.tile([P, 2, C], BF16, tag="pu")
                a_bc = usrc[:, 0:2, :].unsqueeze(2).broadcast_to([P, 2, jj, ii])
                u_bc = usrc[:, 2:4, :].unsqueeze(2).broadcast_to([P, 2, jj, ii])
                nc.vector.tensor_tensor(
                    pa[:, :, :].rearrange("p a (j i) -> p a j i", j=jj), a_bc, e4,
                    op=MULT,
                )
                nc.vector.tensor_tensor(
                    pu[:, :, :].rearrange("p a (j i) -> p a j i", j=jj), u_bc, e4,
                    op=MULT,
                )

                L = psp.tile([P, C], F32, tag="L")
                S = psp.tile([P, C], F32, tag="S")
                nc.tensor.matmul(L[0:36, :], W4z, pa[:, 0, :])
                nc.tensor.matmul(L[32:36, :], W4z[:, 0:4], pa[:, 1, :])
                nc.tensor.matmul(S[0:36, :], W4z, pu[:, 0, :])
                nc.tensor.matmul(S[32:36, :], W4z[:, 0:4], pu[:, 1, :])

                Z = tailp.tile([P, 2, C], BF16, tag="Z")
                nc.scalar.activation(Z[0:36, 0, :], L[0:36, :], EXP)

                wp_q.append((k, Z, S))
                if len(wp_q) > 1:
                    ck = wp_q.pop(0)
                    emit_wprod(ck)
                    nd_q.append(ck)
                if len(nd_q) > 1:
                    emit_numden(nd_q.pop(0))

            while wp_q:
                ck = wp_q.pop(0)
                emit_wprod(ck)
                nd_q.append(ck)
            while nd_q:
                emit_numden(nd_q.pop(0))

            nc.sync.dma_start(y_d[:, :], y_sb[0:16, :])

    # The Tile scheduler round-robins Pool-DMA completion sems over 8 DMASW
    # lanes in ITS final order; each physical sem must stay on ONE SWDGE
    # queue (ucode shadow-sem ring bookkeeping).  Rewrite queue_num from the
    # assigned lane so lane<->queue is consistent by construction.
    for f in nc.m.functions:
        for bb in f.blocks:
            for inst in bb.instructions:
                if type(inst).__name__ == "InstDMAGatherAnt":
                    lane = None
                    si = inst.sync_info
                    for upd in si.on_update if si else []:
                        nm = getattr(upd, "ant_name", "") or ""
                        if nm.startswith("DMASW"):
                            lane = int(nm[5 : nm.index("_")])
                    assert lane is not None, inst.name
                    inst.queue_num = lane % 4

    nc.compile()
    return nc


def pack_tables(taste_emb, attn_emb, movie_emb):
    taste = np.asarray(taste_emb, dtype=np.float32)[:N_ROWS]
    attn = np.asarray(attn_emb, dtype=np.float32)[:N_ROWS]
    mov = np.asarray(movie_emb, dtype=np.float32)

    user_packed = np.concatenate([attn, taste], axis=1)  # [N, 512]
    movie_packed = np.tile(mov, (1, 4))  # [N, 128]
    user_packed = user_packed.astype(ml_dtypes.bfloat16)
    movie_packed = movie_packed.astype(ml_dtypes.bfloat16)

    # dense transposed per-core tables: [128, 4 planes, UPAD]
    userT = np.zeros((N_CORES, P, 4, UPAD), ml_dtypes.bfloat16)
    rows = user_packed.reshape(N_ROWS, 4, P)  # [u, plane, p]
    for r in range(N_CORES):
        blk = rows[r * UPC : (r + 1) * UPC]  # [2500, 4, 128]
        userT[r, :, :, :UPC] = blk.transpose(2, 1, 0)
    return user_packed, movie_packed, userT


def make_consts():
    W4z = np.zeros((P, 36), np.float32)
    for j in range(4):
        W4z[j * 32 : (j + 1) * 32, j] = 1.0
    W5v = np.zeros((P, 16 * ST), np.float32)
    for j in range(ST):
        W5v[[0, 1, 2, 3, 32, 33, 34, 35], 16 * j + j] = 1.0
    ident = np.eye(P, dtype=np.float32)
    return np.concatenate([W4z, W5v, ident], axis=1).astype(ml_dtypes.bfloat16)


def wrap_idx(idx_logical: np.ndarray) -> np.ndarray:
    """dma_gather idx layout for ONE gather: [128, n/16] int16
    (16-partition wrap, replicated x8)."""
    n = idx_logical.shape[0]
    w = idx_logical.astype(np.int16).reshape(n // 16, 16).T  # [16, n/16]
    return np.tile(w, (P // 16, 1))


def layout_core_edges(u_loc, v_loc, eidx):
    """Slot layout for one core.

    Main region: user u gets CU slots; chunk k = u//16 covers users
    16k..16k+16, column c = j*16 + (u%16) holds the user's j-th edge.
    Overflow: groups of GOV edges of one user; group g lives in ov chunk
    g//48 at columns c = j*48 + (g%48).

    Returns (movie_idx [N_CHUNKS*C], ov_user_idx [OV_GROUPS],
             slot_edge [N_CHUNKS*C] with -1 for dummies).
    """
    order = np.argsort(u_loc, kind="stable")
    us, vs, es = u_loc[order], v_loc[order], eidx[order]
    cnt = np.bincount(us, minlength=UPC)
    start = np.concatenate([[0], np.cumsum(cnt)[:-1]])
    rank = np.arange(len(us)) - start[us]

    movie_idx = np.zeros(N_CHUNKS * C, np.int64)
    slot_edge = np.full(N_CHUNKS * C, -1, np.int64)
    ov_user = np.zeros(OV_GROUPS, np.int64)

    main = rank < CU
    um, vm, em, rm = us[main], vs[main], es[main], rank[main]
    slot = (um // UCHUNK) * C + rm * UCHUNK + (um % UCHUNK)
    movie_idx[slot] = vm
    slot_edge[slot] = em

    ex = ~main
    ue, ve, ee, re = us[ex], vs[ex], es[ex], rank[ex] - CU
    # group id: sequential over (user, re//GOV) pairs in sorted order
    gkey = ue * 4096 + re // GOV  # n < 4096*GOV edges/user guaranteed
    uniq, ginv = np.unique(gkey, return_inverse=True)
    ng = len(uniq)
    assert ng <= OV_GROUPS, f"overflow groups {ng} > capacity {OV_GROUPS}"
    ov_user[:ng] = uniq // 4096
    g = ginv
    j = re % GOV
    slot = (MAIN_CHUNKS + g // GPC_OV) * C + j * GPC_OV + (g % GPC_OV)
    movie_idx[slot] = ve
    slot_edge[slot] = ee
    return movie_idx, ov_user, slot_edge


def prepare(edge, taste_emb, attn_emb, movie_emb, user_bias, movie_bias):
    edge = np.asarray(edge)
    u = edge[:, 0].astype(np.int64)
    v = edge[:, 1].astype(np.int64)
    assert edge.shape[0] == B
    assert u.max() < N_ROWS and v.max() < N_ROWS

    user_packed, movie_packed, userT = pack_tables(taste_emb, attn_emb, movie_emb)
    consts = make_consts()

    ub = np.asarray(user_bias, np.float32).reshape(-1)
    mb = np.asarray(movie_bias, np.float32).reshape(-1)
    host_bias = None
    if ub.any() or mb.any():
        host_bias = ub[u] + mb[v]

    in_maps = []
    slot_edge_all = []
    for r in range(N_CORES):
        sel = np.flatnonzero(u // UPC == r)
        movie_idx, ov_user, slot_edge = layout_core_edges(
            u[sel] - r * UPC, v[sel], sel
        )
        slot_edge_all.append(slot_edge)
        ov_user_g = ov_user + r * UPC  # global user row ids
        parts = [
            wrap_idx(movie_idx[gi * GIDX : (gi + 1) * GIDX])
            for gi in range(N_MOV_GATHERS)
        ]
        for oj in range(N_OV_GATHERS):
            blk = np.zeros(OV_IDXPAD, np.int64)
            seg = ov_user_g[
                oj * OV_PER_GATHER * GPC_OV : (oj + 1) * OV_PER_GATHER * GPC_OV
            ]
            blk[: len(seg)] = seg
            parts.append(wrap_idx(blk))
        idx_uv = np.concatenate(parts, axis=1)
        assert idx_uv.shape == (P, IDX_COLS), idx_uv.shape
        in_maps.append(
            {
                "userT": userT[r].reshape(P, 4 * UPAD),
                "user_packed": user_packed,
                "movie_packed": movie_packed,
                "idx_uv": idx_uv,
                "consts": consts,
            }
        )
    filled = sum(int((se >= 0).sum()) for se in slot_edge_all)
    assert filled == B, filled
    return in_maps, (slot_edge_all, host_bias)


_NC_CACHE: list = []


def run(in_maps, **kwargs):
    if not _NC_CACHE:
        _NC_CACHE.append(build_nc())
    return run_bass_kernel_spmd(
        _NC_CACHE[0], in_maps, core_ids=list(range(N_CORES)), **kwargs
    )


def unscatter(res, aux):
    slot_edge_all, host_bias = aux
    y = np.empty(B, dtype=np.float32)
    for r in range(N_CORES):
        yc = res.results[r]["y"]  # [16, NCOL_OUT]
        se = slot_edge_all[r]  # [N_CHUNKS*C]
        s = np.flatnonzero(se >= 0)
        k, c = s // C, s % C
        y[se[s]] = yc[k % ST, (k // ST) * C + c]
    if host_bias is not None:
        y = y + host_bias
    return y


def kernel(edge, taste_emb, attn_emb, movie_emb, user_bias, movie_bias):
    in_maps, aux = prepare(
        edge, taste_emb, attn_emb, movie_emb, user_bias, movie_bias
    )
    res = run(in_maps)
    return unscatter(res, aux)


# revision 15
# speedup vs baseline: 2.0650x; 1.0938x over previous
"""Trainium2 Bass kernel for the mixture-of-tastes edge scoring model.

y[b] = sum_m softmax_m(A[u_b] @ e[v_b]) * (U[u_b] @ e[v_b]) + ub[u_b] + mb[v_b]

The kernel is gather-descriptor-bound on TRN2 (the Q7 SWDGE generates
descriptors at ~8-10 ns each), so the layout is built to minimize
descriptor count:

- Edges are partitioned across the 8 cores BY USER RANGE (user u goes to
  core u // 2500), so each user's ~26 edges land on one core.  Each core's
  edges are grouped by user into groups of G=8 slots (padded with dummy
  slots), so ONE user-row gather descriptor serves 8 edges.
- Movie rows are gathered per slot (unavoidable: 1 descriptor each).
- Group j maps to (partition j%128, output column block j//128); slot s of
  group j is output element [j%128, (j//128)*8 + s].  The host keeps a
  slot->edge map and unscatters at the end (dummy slots dropped).

Tables are packed on the host into gather-friendly bf16 rows (bf16 also
gives the DVE its 2x 16-bit mode):

  user_packed[u]  = [attn(8x32) | taste'(8x34) | pad]  (640 bf16 = 1280 B)
      taste'[m] = [taste[m] (32) | user_bias[u] | 1.0]
  movie_packed[v] = [e (32) | 1.0 | mb | pad]          (128 bf16 = 256 B)

With e'' = movie_packed[v][0:34] = [e, 1, mb], the fold
  U'_m . e'' = U_m . e + ub + mb
adds (ub+mb) to every score; softmax weights sum to 1, so the output gets
+(ub+mb) with no separate bias gather.  Softmax is computed without max
subtraction (logits are O(1e-2) here; exp cannot overflow).

Per 1024-slot chunk: one movie dma_gather + DVE broadcast-multiply
(user rows broadcast over the 8 slots of their group) + 3D-AP reduces,
ACT exp, DVE weighted combine.  One 1024-group user dma_gather feeds 8
chunks.
"""

import sys

sys.path.insert(0, "/opt/trn_rl_repo")

import ml_dtypes
import numpy as np

import concourse.bacc as bacc
import concourse.bass as bass
import concourse.mybir as mybir
from concourse.bass_utils import run_bass_kernel_spmd
from concourse.tile import TileContext

# Problem constants (nn_MoT_43533788512463)
B = 524288
N_CORES = 8
M, K = 8, 32
N_ROWS = 20000  # edge indices are randint(0, 20000) per the spec
UPC = N_ROWS // N_CORES  # users per core (u-range partitioning)
G = 8  # slots (edges) per user group
UROW = 640  # packed user row bf16: 256 attn + 272 taste' + 112 pad
VROW = 128  # packed movie row bf16: 32 e + 1.0 + mb + 94 pad
P = 128
CHUNK = 1024  # slots per movie gather / compute chunk
NBLK = CHUNK // P  # 8 column blocks per chunk

# Per-core slot capacity.  Expected need: 2500 users x E[ceil(n/8)] groups
# ~= 9570 +- 25; 9728 groups (76 chunks) is >6 sigma of slack.
N_CHUNKS = 76
CAP = N_CHUNKS * CHUNK  # 77824 slots
GPC = 512  # groups per user gather (512 groups = 4 chunks)
SC_CHUNKS = GPC * G // CHUNK  # 4 chunks per user super-chunk
N_SC = N_CHUNKS // SC_CHUNKS  # 19 user gathers
COLS = CAP // P  # 608 output columns per partition

BF16 = mybir.dt.bfloat16
F32 = mybir.dt.float32
I16 = mybir.dt.int16
MULT = mybir.AluOpType.mult
ADD = mybir.AluOpType.add
AX_X = mybir.AxisListType.X


def build_nc() -> bass.Bass:
    """One NeuronCore's program; SPMD across cores with different inputs."""
    nc = bacc.Bacc("TRN2", debug=False, num_swdge_queues=4)
    user_d = nc.dram_tensor("user_packed", [N_ROWS, UROW], BF16, kind="ExternalInput")
    movie_d = nc.dram_tensor("movie_packed", [N_ROWS, VROW], BF16, kind="ExternalInput")
    # user idx: N_SC gathers x (GPC/16) cols; movie idx: N_CHUNKS x (CHUNK/16)
    uw, vw = GPC // 16, CHUNK // 16
    idx_d = nc.dram_tensor(
        "idx_uv", [P, N_SC * uw + N_CHUNKS * vw], I16, kind="ExternalInput"
    )
    y_d = nc.dram_tensor("y", [P, COLS], F32, kind="ExternalOutput")

    with TileContext(nc) as tc:
        with (
            tc.tile_pool(name="persist", bufs=1) as pp,
            tc.tile_pool(name="io", bufs=4) as iop,
            tc.tile_pool(name="mid", bufs=3) as midp,
        ):
            idxs = pp.tile([P, N_SC * uw + N_CHUNKS * vw], I16)
            nc.sync.dma_start(idxs[:, :], idx_d[:, :])
            ysb = pp.tile([P, COLS], F32)

            for sc in range(N_SC):
                us = iop.tile([P, SC_CHUNKS, UROW], BF16, tag="us")
                usl = idxs[:, sc * uw : (sc + 1) * uw]
                nc.gpsimd.dma_gather(
                    us[:, :, :], user_d[:, :], usl, GPC, GPC, UROW, queue_num=0
                )
                for cc2 in range(SC_CHUNKS // 2):
                    # one 2048-idx movie gather feeds two compute chunks
                    mv2 = iop.tile([P, 2, NBLK, VROW], BF16, tag="mv2")
                    cpair = sc * SC_CHUNKS + cc2 * 2
                    vsl = idxs[
                        :,
                        N_SC * uw + cpair * vw : N_SC * uw + (cpair + 2) * vw,
                    ]
                    nc.gpsimd.dma_gather(
                        mv2[:, :, :, :].rearrange("p a b v -> p (a b) v"),
                        movie_d[:, :],
                        vsl,
                        2 * CHUNK,
                        2 * CHUNK,
                        VROW,
                        single_packet=False,
                        queue_num=1 + (sc * (SC_CHUNKS // 2) + cc2) % 3,
                    )
                    yield_chunks = [
                        (cc2 * 2, mv2[:, 0, :, :]),
                        (cc2 * 2 + 1, mv2[:, 1, :, :]),
                    ]
                    for cc, mv in yield_chunks:
                        c = sc * SC_CHUNKS + cc

                        # group's user row broadcast over its 8 slots (dim 1);
                        # slot's movie row broadcast over the 8 tastes (dim 2)
                        a4 = (
                            us[:, cc, 0:256]
                            .rearrange("p (m k) -> p m k", m=M)
                            .unsqueeze(1)
                            .broadcast_to([P, NBLK, M, K])
                        )
                        u4 = (
                            us[:, cc, 256:528]
                            .rearrange("p (m k) -> p m k", m=M)
                            .unsqueeze(1)
                            .broadcast_to([P, NBLK, M, K + 2])
                        )
                        e32 = (
                            mv[:, :, 0:K].unsqueeze(2).broadcast_to([P, NBLK, M, K])
                        )
                        e34 = (
                            mv[:, :, 0 : K + 2]
                            .unsqueeze(2)
                            .broadcast_to([P, NBLK, M, K + 2])
                        )

                        prod_a = midp.tile([P, NBLK, M, K], BF16, tag="prod_a")
                        prod_u = midp.tile([P, NBLK, M, K + 2], BF16, tag="prod_u")
                        half_a = midp.tile([P, NBLK, M, K // 2], BF16, tag="half_a")
                        half_u = midp.tile([P, NBLK, M, K // 2 + 1], BF16, tag="half_u")
                        logits = midp.tile([P, NBLK, M], F32, tag="logits")
                        scores = midp.tile([P, NBLK, M], F32, tag="scores")
                        exps = midp.tile([P, NBLK, M], F32, tag="exps")
                        wprod = midp.tile([P, NBLK, M], F32, tag="wprod")
                        num_t = midp.tile([P, NBLK], F32, tag="num_t")
                        den_t = midp.tile([P, NBLK], F32, tag="den_t")
                        rden_t = midp.tile([P, NBLK], F32, tag="rden_t")

                        # mul at bf16 2x; fold k in half with a bf16 add (2x)
                        # before tensor_reduce, which only has a 1x uop
                        nc.vector.tensor_tensor(prod_a[:, :, :, :], a4, e32, op=MULT)
                        nc.vector.tensor_tensor(
                            half_a[:, :, :, :],
                            prod_a[:, :, :, 0 : K // 2],
                            prod_a[:, :, :, K // 2 : K],
                            op=ADD,
                        )
                        nc.vector.tensor_reduce(
                            logits[:, :, :], half_a[:, :, :, :], AX_X, ADD
                        )
                        nc.vector.tensor_tensor(prod_u[:, :, :, :], u4, e34, op=MULT)
                        nc.vector.tensor_tensor(
                            half_u[:, :, :, :],
                            prod_u[:, :, :, 0 : K // 2 + 1],
                            prod_u[:, :, :, K // 2 + 1 : K + 2],
                            op=ADD,
                        )
                        nc.vector.tensor_reduce(
                            scores[:, :, :], half_u[:, :, :, :], AX_X, ADD
                        )
                        nc.scalar.activation(
                            exps[:, :, :],
                            logits[:, :, :],
                            mybir.ActivationFunctionType.Exp,
                        )
                        nc.vector.tensor_tensor(
                            wprod[:, :, :], exps[:, :, :], scores[:, :, :], op=MULT
                        )
                        nc.vector.tensor_reduce(num_t[:, :], wprod[:, :, :], AX_X, ADD)
                        nc.vector.tensor_reduce(den_t[:, :], exps[:, :, :], AX_X, ADD)
                        nc.vector.reciprocal(rden_t[:, :], den_t[:, :])
                        nc.vector.tensor_tensor(
                            ysb[:, c * NBLK : (c + 1) * NBLK],
                            num_t[:, :],
                            rden_t[:, :],
                            op=MULT,
                        )

            nc.sync.dma_start(y_d[:, :], ysb[:, :])

    # Each physical DMASW completion sem must stay on ONE SWDGE queue
    # (ucode shadow-sem ring bookkeeping).  The Tile scheduler round-robins
    # Pool-DMA sems over 8 DMASW lanes in its final order; derive queue_num
    # from the assigned lane so lane<->queue is consistent by construction.
    for f in nc.m.functions:
        for bb in f.blocks:
            for inst in bb.instructions:
                if type(inst).__name__ == "InstDMAGatherAnt":
                    lane = None
                    si = inst.sync_info
                    for upd in si.on_update if si else []:
                        nm = getattr(upd, "ant_name", "") or ""
                        if nm.startswith("DMASW"):
                            lane = int(nm[5 : nm.index("_")])
                    assert lane is not None, inst.name
                    inst.queue_num = lane % 4

    nc.compile()
    return nc


def pack_tables(taste_emb, attn_emb, movie_emb, user_bias, movie_bias):
    taste_emb = np.asarray(taste_emb, dtype=np.float32)
    attn_emb = np.asarray(attn_emb, dtype=np.float32)
    movie_emb = np.asarray(movie_emb, dtype=np.float32)
    user_bias = np.asarray(user_bias, dtype=np.float32)
    movie_bias = np.asarray(movie_bias, dtype=np.float32)

    nr = N_ROWS
    ublk = np.zeros((nr, M, K + 2), np.float32)
    ublk[:, :, :K] = taste_emb[:nr].reshape(nr, M, K)
    ublk[:, :, K] = user_bias[:nr, 0][:, None]
    ublk[:, :, K + 1] = 1.0
    user_packed = np.zeros((nr, UROW), np.float32)
    user_packed[:, 0:256] = attn_emb[:nr]
    user_packed[:, 256:528] = ublk.reshape(nr, 272)

    nm = movie_emb.shape[0]
    assert nm <= N_ROWS
    movie_packed = np.zeros((N_ROWS, VROW), np.float32)
    movie_packed[:nm, :K] = movie_emb
    movie_packed[:nm, K] = 1.0
    movie_packed[:nm, K + 1] = movie_bias[:, 0]
    return (
        user_packed.astype(ml_dtypes.bfloat16),
        movie_packed.astype(ml_dtypes.bfloat16),
    )


def wrap_idx(idx_logical: np.ndarray) -> np.ndarray:
    """dma_gather idx layout for ONE gather: [128, n/16] int16
    (16-partition wrap, replicated x8)."""
    n = idx_logical.shape[0]
    w = idx_logical.astype(np.int16).reshape(n // 16, 16).T  # [16, n/16]
    return np.tile(w, (P // 16, 1))


def group_core_edges(u, v, eidx):
    """Group one core's edges by user into G-slot groups.

    Returns (group_user [NGROUPS], slot_v [NGROUPS, G], slot_edge
    [NGROUPS, G] with -1 for dummy slots).  Group j is computed by
    (partition j%128, chunk j//128).
    """
    ngroups = CAP // G
    order = np.argsort(u, kind="stable")
    u_s, v_s, e_s = u[order], v[order], eidx[order]
    # segment boundaries per user
    bounds = np.flatnonzero(np.diff(u_s)) + 1
    starts = np.concatenate([[0], bounds])
    ends = np.concatenate([bounds, [len(u_s)]])

    group_user = np.full(ngroups, u[0] if len(u) else 0, dtype=np.int64)
    slot_v = np.zeros((ngroups, G), dtype=np.int64)
    slot_edge = np.full((ngroups, G), -1, dtype=np.int64)
    gj = 0
    for s, e in zip(starts, ends):
        for base in range(s, e, G):
            take = min(G, e - base)
            assert gj < ngroups, "CAP too small for this edge distribution"
            group_user[gj] = u_s[s]
            slot_v[gj, :take] = v_s[base : base + take]
            slot_edge[gj, :take] = e_s[base : base + take]
            gj += 1
    return group_user, slot_v, slot_edge


def prepare(edge, taste_emb, attn_emb, movie_emb, user_bias, movie_bias):
    edge = np.asarray(edge)
    u = edge[:, 0].astype(np.int64)
    v = edge[:, 1].astype(np.int64)
    b = edge.shape[0]
    assert b == B
    assert u.max() < N_ROWS and v.max() < N_ROWS

    user_packed, movie_packed = pack_tables(
        taste_emb, attn_emb, movie_emb, user_bias, movie_bias
    )

    core_of = u // UPC  # user-range partitioning
    uw, vw = GPC // 16, CHUNK // 16

    in_maps = []
    slot_edge_all = []
    for r in range(N_CORES):
        sel = np.flatnonzero(core_of == r)
        gu, sv, se = group_core_edges(u[sel], v[sel], sel)
        slot_edge_all.append(se)

        # group j -> (partition j%128, chunk j//128).  User gather sc covers
        # groups j in [sc*GPC, (sc+1)*GPC): logical gather position i ->
        # partition i%128, block i//128 = cc; so position i = group
        # (sc*G + i//128)*128 + i%128.
        gu_by_chunkpart = gu.reshape(N_CHUNKS, P)  # [chunk, partition]
        uparts = []
        for sc in range(N_SC):
            blk = gu_by_chunkpart[
                sc * SC_CHUNKS : (sc + 1) * SC_CHUNKS
            ]  # [SC_CHUNKS(cc), P]
            uparts.append(wrap_idx(blk.reshape(-1)))
        # movie gather for chunk c: position i -> partition i%128, slot i//128
        # = slot s of group j = c*128 + i%128
        sv_by = sv.reshape(N_CHUNKS, P, G)  # [chunk, partition(j%128), slot]
        vparts = []
        for c in range(N_CHUNKS):
            vparts.append(wrap_idx(sv_by[c].T.reshape(-1)))  # (s p) order
        idx_uv = np.concatenate(uparts + vparts, axis=1)
        assert idx_uv.shape == (P, N_SC * uw + N_CHUNKS * vw)
        in_maps.append(
            {
                "user_packed": user_packed,
                "movie_packed": movie_packed,
                "idx_uv": idx_uv,
            }
        )
    return in_maps, slot_edge_all


_NC_CACHE: list = []


def run(in_maps, **kwargs):
    if not _NC_CACHE:
        _NC_CACHE.append(build_nc())
    return run_bass_kernel_spmd(
        _NC_CACHE[0], in_maps, core_ids=list(range(N_CORES)), **kwargs
    )


def unscatter(res, slot_edge_all):
    y = np.empty(B, dtype=np.float32)
    filled = 0
    for r in range(N_CORES):
        yc = res.results[r]["y"]  # [P, COLS]
        se = slot_edge_all[r]  # [NGROUPS, G]
        # slot s of group j -> yc[j%128, (j//128)*G + s]
        ngroups = se.shape[0]
        j = np.arange(ngroups)
        part = (j % P)[:, None]
        col = ((j // P) * G)[:, None] + np.arange(G)[None, :]
        vals = yc[part, col]  # [NGROUPS, G]
        mask = se >= 0
        y[se[mask]] = vals[mask]
        filled += int(mask.sum())
    assert filled == B
    return y


def kernel(edge, taste_emb, attn_emb, movie_emb, user_bias, movie_bias):
    in_maps, slot_edge_all = prepare(
        edge, taste_emb, attn_emb, movie_emb, user_bias, movie_bias
    )
    res = run(in_maps)
    return unscatter(res, slot_edge_all)



# revision 16
# speedup vs baseline: 2.1117x; 1.0226x over previous
"""Trainium2 Bass kernel for the mixture-of-tastes edge scoring model.

y[b] = sum_m softmax_m(A[u_b] @ e[v_b]) * (U[u_b] @ e[v_b]) + ub[u_b] + mb[v_b]

The kernel is gather-descriptor-bound on TRN2 (the Q7 SWDGE generates
descriptors at ~8-10 ns each), so the layout is built to minimize
descriptor count:

- Edges are partitioned across the 8 cores BY USER RANGE (user u goes to
  core u // 2500), so each user's ~26 edges land on one core.  Each core's
  edges are grouped by user into groups of G=8 slots (padded with dummy
  slots), so ONE user-row gather descriptor serves 8 edges.
- Movie rows are gathered per slot (unavoidable: 1 descriptor each).
- Group j maps to (partition j%128, output column block j//128); slot s of
  group j is output element [j%128, (j//128)*8 + s].  The host keeps a
  slot->edge map and unscatters at the end (dummy slots dropped).

Tables are packed on the host into gather-friendly bf16 rows (bf16 also
gives the DVE its 2x 16-bit mode):

  user_packed[u]  = [attn(8x32) | taste'(8x34) | pad]  (640 bf16 = 1280 B)
      taste'[m] = [taste[m] (32) | user_bias[u] | 1.0]
  movie_packed[v] = [e (32) | 1.0 | mb | pad]          (128 bf16 = 256 B)

With e'' = movie_packed[v][0:34] = [e, 1, mb], the fold
  U'_m . e'' = U_m . e + ub + mb
adds (ub+mb) to every score; softmax weights sum to 1, so the output gets
+(ub+mb) with no separate bias gather.  Softmax is computed without max
subtraction (logits are O(1e-2) here; exp cannot overflow).

Per 1024-slot chunk: one movie dma_gather + DVE broadcast-multiply
(user rows broadcast over the 8 slots of their group) + 3D-AP reduces,
ACT exp, DVE weighted combine.  One 1024-group user dma_gather feeds 8
chunks.
"""

import sys

sys.path.insert(0, "/opt/trn_rl_repo")

import ml_dtypes
import numpy as np

import concourse.bacc as bacc
import concourse.bass as bass
import concourse.mybir as mybir
from concourse.bass_utils import run_bass_kernel_spmd
from concourse.tile import TileContext

# Problem constants (nn_MoT_43533788512463)
B = 524288
N_CORES = 8
M, K = 8, 32
N_ROWS = 20000  # edge indices are randint(0, 20000) per the spec
UPC = N_ROWS // N_CORES  # users per core (u-range partitioning)
G = 8  # slots (edges) per user group
UROW = 640  # packed user row bf16: 256 attn + 272 taste' + 112 pad
VROW = 128  # packed movie row bf16: 32 e + 1.0 + mb + 94 pad
P = 128
CHUNK = 1024  # slots per movie gather / compute chunk
NBLK = CHUNK // P  # 8 column blocks per chunk

# Per-core slot capacity.  Expected need: 2500 users x E[ceil(n/8)] groups
# ~= 9570 +- 25; 9728 groups (76 chunks) is >6 sigma of slack.
N_CHUNKS = 76
CAP = N_CHUNKS * CHUNK  # 77824 slots
GPC = 512  # groups per user gather (512 groups = 4 chunks)
SC_CHUNKS = GPC * G // CHUNK  # 4 chunks per user super-chunk
N_SC = N_CHUNKS // SC_CHUNKS  # 19 user gathers
COLS = CAP // P  # 608 output columns per partition

BF16 = mybir.dt.bfloat16
F32 = mybir.dt.float32
I16 = mybir.dt.int16
MULT = mybir.AluOpType.mult
ADD = mybir.AluOpType.add
AX_X = mybir.AxisListType.X


def build_nc() -> bass.Bass:
    """One NeuronCore's program; SPMD across cores with different inputs."""
    nc = bacc.Bacc("TRN2", debug=False, num_swdge_queues=4)
    user_d = nc.dram_tensor("user_packed", [N_ROWS, UROW], BF16, kind="ExternalInput")
    movie_d = nc.dram_tensor("movie_packed", [N_ROWS, VROW], BF16, kind="ExternalInput")
    # user idx: N_SC gathers x (GPC/16) cols; movie idx: N_CHUNKS x (CHUNK/16)
    uw, vw = GPC // 16, CHUNK // 16
    idx_d = nc.dram_tensor(
        "idx_uv", [P, N_SC * uw + N_CHUNKS * vw], I16, kind="ExternalInput"
    )
    y_d = nc.dram_tensor("y", [P, COLS], F32, kind="ExternalOutput")

    with TileContext(nc) as tc:
        with (
            tc.tile_pool(name="persist", bufs=1) as pp,
            tc.tile_pool(name="io", bufs=7) as iop,
            tc.tile_pool(name="mid", bufs=4) as midp,
        ):
            idxs = pp.tile([P, N_SC * uw + N_CHUNKS * vw], I16)
            nc.sync.dma_start(idxs[:, :], idx_d[:, :])
            ysb = pp.tile([P, COLS], F32)

            for sc in range(N_SC):
                us = iop.tile([P, SC_CHUNKS, UROW], BF16, tag="us")
                usl = idxs[:, sc * uw : (sc + 1) * uw]
                nc.gpsimd.dma_gather(
                    us[:, :, :], user_d[:, :], usl, GPC, GPC, UROW, queue_num=0
                )
                for cc2 in range(SC_CHUNKS // 2):
                    # one 2048-idx movie gather feeds two compute chunks
                    mv2 = iop.tile([P, 2, NBLK, VROW], BF16, tag="mv2")
                    cpair = sc * SC_CHUNKS + cc2 * 2
                    vsl = idxs[
                        :,
                        N_SC * uw + cpair * vw : N_SC * uw + (cpair + 2) * vw,
                    ]
                    nc.gpsimd.dma_gather(
                        mv2[:, :, :, :].rearrange("p a b v -> p (a b) v"),
                        movie_d[:, :],
                        vsl,
                        2 * CHUNK,
                        2 * CHUNK,
                        VROW,
                        single_packet=False,
                        queue_num=1 + (sc * (SC_CHUNKS // 2) + cc2) % 3,
                    )
                    yield_chunks = [
                        (cc2 * 2, mv2[:, 0, :, :]),
                        (cc2 * 2 + 1, mv2[:, 1, :, :]),
                    ]
                    for cc, mv in yield_chunks:
                        c = sc * SC_CHUNKS + cc

                        # group's user row broadcast over its 8 slots (dim 1);
                        # slot's movie row broadcast over the 8 tastes (dim 2)
                        a4 = (
                            us[:, cc, 0:256]
                            .rearrange("p (m k) -> p m k", m=M)
                            .unsqueeze(1)
                            .broadcast_to([P, NBLK, M, K])
                        )
                        u4 = (
                            us[:, cc, 256:528]
                            .rearrange("p (m k) -> p m k", m=M)
                            .unsqueeze(1)
                            .broadcast_to([P, NBLK, M, K + 2])
                        )
                        e32 = (
                            mv[:, :, 0:K].unsqueeze(2).broadcast_to([P, NBLK, M, K])
                        )
                        e34 = (
                            mv[:, :, 0 : K + 2]
                            .unsqueeze(2)
                            .broadcast_to([P, NBLK, M, K + 2])
                        )

                        prod_a = midp.tile([P, NBLK, M, K], BF16, tag="prod_a")
                        prod_u = midp.tile([P, NBLK, M, K + 2], BF16, tag="prod_u")
                        half_a = midp.tile([P, NBLK, M, K // 2], BF16, tag="half_a")
                        half_u = midp.tile([P, NBLK, M, K // 2 + 1], BF16, tag="half_u")
                        logits = midp.tile([P, NBLK, M], F32, tag="logits")
                        scores = midp.tile([P, NBLK, M], F32, tag="scores")
                        exps = midp.tile([P, NBLK, M], F32, tag="exps")
                        wprod = midp.tile([P, NBLK, M], F32, tag="wprod")
                        num_t = midp.tile([P, NBLK], F32, tag="num_t")
                        den_t = midp.tile([P, NBLK], F32, tag="den_t")
                        rden_t = midp.tile([P, NBLK], F32, tag="rden_t")

                        # mul at bf16 2x; fold k in half with a bf16 add (2x)
                        # before tensor_reduce, which only has a 1x uop
                        nc.vector.tensor_tensor(prod_a[:, :, :, :], a4, e32, op=MULT)
                        nc.vector.tensor_tensor(
                            half_a[:, :, :, :],
                            prod_a[:, :, :, 0 : K // 2],
                            prod_a[:, :, :, K // 2 : K],
                            op=ADD,
                        )
                        nc.vector.tensor_reduce(
                            logits[:, :, :], half_a[:, :, :, :], AX_X, ADD
                        )
                        nc.vector.tensor_tensor(prod_u[:, :, :, :], u4, e34, op=MULT)
                        nc.vector.tensor_tensor(
                            half_u[:, :, :, :],
                            prod_u[:, :, :, 0 : K // 2 + 1],
                            prod_u[:, :, :, K // 2 + 1 : K + 2],
                            op=ADD,
                        )
                        nc.vector.tensor_reduce(
                            scores[:, :, :], half_u[:, :, :, :], AX_X, ADD
                        )
                        nc.scalar.activation(
                            exps[:, :, :],
                            logits[:, :, :],
                            mybir.ActivationFunctionType.Exp,
                        )
                        nc.vector.tensor_tensor(
                            wprod[:, :, :], exps[:, :, :], scores[:, :, :], op=MULT
                        )
                        nc.vector.tensor_reduce(num_t[:, :], wprod[:, :, :], AX_X, ADD)
                        nc.vector.tensor_reduce(den_t[:, :], exps[:, :, :], AX_X, ADD)
                        nc.vector.reciprocal(rden_t[:, :], den_t[:, :])
                        nc.vector.tensor_tensor(
                            ysb[:, c * NBLK : (c + 1) * NBLK],
                            num_t[:, :],
                            rden_t[:, :],
                            op=MULT,
                        )

            nc.sync.dma_start(y_d[:, :], ysb[:, :])

    # Each physical DMASW completion sem must stay on ONE SWDGE queue
    # (ucode shadow-sem ring bookkeeping).  The Tile scheduler round-robins
    # Pool-DMA sems over 8 DMASW lanes in its final order; derive queue_num
    # from the assigned lane so lane<->queue is consistent by construction.
    for f in nc.m.functions:
        for bb in f.blocks:
            for inst in bb.instructions:
                if type(inst).__name__ == "InstDMAGatherAnt":
                    lane = None
                    si = inst.sync_info
                    for upd in si.on_update if si else []:
                        nm = getattr(upd, "ant_name", "") or ""
                        if nm.startswith("DMASW"):
                            lane = int(nm[5 : nm.index("_")])
                    assert lane is not None, inst.name
                    inst.queue_num = lane % 4

    nc.compile()
    return nc


def pack_tables(taste_emb, attn_emb, movie_emb, user_bias, movie_bias):
    taste_emb = np.asarray(taste_emb, dtype=np.float32)
    attn_emb = np.asarray(attn_emb, dtype=np.float32)
    movie_emb = np.asarray(movie_emb, dtype=np.float32)
    user_bias = np.asarray(user_bias, dtype=np.float32)
    movie_bias = np.asarray(movie_bias, dtype=np.float32)

    nr = N_ROWS
    ublk = np.zeros((nr, M, K + 2), np.float32)
    ublk[:, :, :K] = taste_emb[:nr].reshape(nr, M, K)
    ublk[:, :, K] = user_bias[:nr, 0][:, None]
    ublk[:, :, K + 1] = 1.0
    user_packed = np.zeros((nr, UROW), np.float32)
    user_packed[:, 0:256] = attn_emb[:nr]
    user_packed[:, 256:528] = ublk.reshape(nr, 272)

    nm = movie_emb.shape[0]
    assert nm <= N_ROWS
    movie_packed = np.zeros((N_ROWS, VROW), np.float32)
    movie_packed[:nm, :K] = movie_emb
    movie_packed[:nm, K] = 1.0
    movie_packed[:nm, K + 1] = movie_bias[:, 0]
    return (
        user_packed.astype(ml_dtypes.bfloat16),
        movie_packed.astype(ml_dtypes.bfloat16),
    )


def wrap_idx(idx_logical: np.ndarray) -> np.ndarray:
    """dma_gather idx layout for ONE gather: [128, n/16] int16
    (16-partition wrap, replicated x8)."""
    n = idx_logical.shape[0]
    w = idx_logical.astype(np.int16).reshape(n // 16, 16).T  # [16, n/16]
    return np.tile(w, (P // 16, 1))


def group_core_edges(u, v, eidx):
    """Group one core's edges by user into G-slot groups.

    Returns (group_user [NGROUPS], slot_v [NGROUPS, G], slot_edge
    [NGROUPS, G] with -1 for dummy slots).  Group j is computed by
    (partition j%128, chunk j//128).
    """
    ngroups = CAP // G
    order = np.argsort(u, kind="stable")
    u_s, v_s, e_s = u[order], v[order], eidx[order]
    # segment boundaries per user
    bounds = np.flatnonzero(np.diff(u_s)) + 1
    starts = np.concatenate([[0], bounds])
    ends = np.concatenate([bounds, [len(u_s)]])

    group_user = np.full(ngroups, u[0] if len(u) else 0, dtype=np.int64)
    slot_v = np.zeros((ngroups, G), dtype=np.int64)
    slot_edge = np.full((ngroups, G), -1, dtype=np.int64)
    gj = 0
    for s, e in zip(starts, ends):
        for base in range(s, e, G):
            take = min(G, e - base)
            assert gj < ngroups, "CAP too small for this edge distribution"
            group_user[gj] = u_s[s]
            slot_v[gj, :take] = v_s[base : base + take]
            slot_edge[gj, :take] = e_s[base : base + take]
            gj += 1
    return group_user, slot_v, slot_edge


def prepare(edge, taste_emb, attn_emb, movie_emb, user_bias, movie_bias):
    edge = np.asarray(edge)
    u = edge[:, 0].astype(np.int64)
    v = edge[:, 1].astype(np.int64)
    b = edge.shape[0]
    assert b == B
    assert u.max() < N_ROWS and v.max() < N_ROWS

    user_packed, movie_packed = pack_tables(
        taste_emb, attn_emb, movie_emb, user_bias, movie_bias
    )

    core_of = u // UPC  # user-range partitioning
    uw, vw = GPC // 16, CHUNK // 16

    in_maps = []
    slot_edge_all = []
    for r in range(N_CORES):
        sel = np.flatnonzero(core_of == r)
        gu, sv, se = group_core_edges(u[sel], v[sel], sel)
        slot_edge_all.append(se)

        # group j -> (partition j%128, chunk j//128).  User gather sc covers
        # groups j in [sc*GPC, (sc+1)*GPC): logical gather position i ->
        # partition i%128, block i//128 = cc; so position i = group
        # (sc*G + i//128)*128 + i%128.
        gu_by_chunkpart = gu.reshape(N_CHUNKS, P)  # [chunk, partition]
        uparts = []
        for sc in range(N_SC):
            blk = gu_by_chunkpart[
                sc * SC_CHUNKS : (sc + 1) * SC_CHUNKS
            ]  # [SC_CHUNKS(cc), P]
            uparts.append(wrap_idx(blk.reshape(-1)))
        # movie gather for chunk c: position i -> partition i%128, slot i//128
        # = slot s of group j = c*128 + i%128
        sv_by = sv.reshape(N_CHUNKS, P, G)  # [chunk, partition(j%128), slot]
        vparts = []
        for c in range(N_CHUNKS):
            vparts.append(wrap_idx(sv_by[c].T.reshape(-1)))  # (s p) order
        idx_uv = np.concatenate(uparts + vparts, axis=1)
        assert idx_uv.shape == (P, N_SC * uw + N_CHUNKS * vw)
        in_maps.append(
            {
                "user_packed": user_packed,
                "movie_packed": movie_packed,
                "idx_uv": idx_uv,
            }
        )
    return in_maps, slot_edge_all


_NC_CACHE: list = []


def run(in_maps, **kwargs):
    if not _NC_CACHE:
        _NC_CACHE.append(build_nc())
    return run_bass_kernel_spmd(
        _NC_CACHE[0], in_maps, core_ids=list(range(N_CORES)), **kwargs
    )


def unscatter(res, slot_edge_all):
    y = np.empty(B, dtype=np.float32)
    filled = 0
    for r in range(N_CORES):
        yc = res.results[r]["y"]  # [P, COLS]
        se = slot_edge_all[r]  # [NGROUPS, G]
        # slot s of group j -> yc[j%128, (j//128)*G + s]
        ngroups = se.shape[0]
        j = np.arange(ngroups)
        part = (j % P)[:, None]
        col = ((j // P) * G)[:, None] + np.arange(G)[None, :]
        vals = yc[part, col]  # [NGROUPS, G]
        mask = se >= 0
        y[se[mask]] = vals[mask]
        filled += int(mask.sum())
    assert filled == B
    return y


def kernel(edge, taste_emb, attn_emb, movie_emb, user_bias, movie_bias):
    in_maps, slot_edge_all = prepare(
        edge, taste_emb, attn_emb, movie_emb, user_bias, movie_bias
    )
    res = run(in_maps)
    return unscatter(res, slot_edge_all)

